# revision 62
# baseline (speedup 1.0000x reference)
"""Trainium2 Bass kernel for nn_Attention_30562987278646.

Sharding: 8 cores = 4 batches x 2 head-groups (4 heads each).
Per core: LN(q/k/v) -> project -> score matrices -> out = S @ f_v ->
partial @ W_out rows. Host sums the 2 head-group partials per batch.

Key identities / layout:
 - LN applied as one fused activation (x*r + (-mu*r)) per n-tile; ln_g
   folded into W on host.
 - cov term: qc . kc == fq . kc (centering q is free), and kc = C f_k
   with C = I - 1/64 folded into the k-side weights on host. So the
   score matmul is ONE K=128 matmul per (m-tile, n-chunk):
     rows 0:64   kc_h      x  fqc_h (= cov_w/64 * fq)
     rows 64:128 fkn_h     x  fqn_h (= cos_w * fq / qn)
 - var term: relu(1-cos)=1-cos (GAMMA=1, |cos|<=1), mean_m(1-cos) gives
   a per-n row vr; its contribution to the output is rank-1
   (vr[n] * colsum_m(f_v)[e]) and is added in the out-stage PSUM
   accumulation as one K=1 matmul per (head, n-chunk).
"""

import sys
import numpy as np
import ml_dtypes

for _p in ("/opt/trn_rl_repo", "/root/.axon_site/_ro/trn_rl_repo"):
    if _p not in sys.path:
        sys.path.append(_p)

HEADS = 8
DIM_HEAD = 64
LN_EPS = 1e-5
B, N, DIM = 4, 1024, 512
HG = 2                      # head groups (shards along heads)
HPG = HEADS // HG           # heads per group = 4
IG = HPG * DIM_HEAD         # inner dim per group = 256
NT = N // 128               # 8 n-tiles
NC = N // 512               # 2 n-chunks
CC = DIM // 128             # 4 c-chunks


def _build_nc(cos_w: float, cov_w: float, var_w: float, has_bias: bool):
    import concourse.bass as bass
    import concourse.bacc as bacc
    import concourse.tile as tile
    from concourse import mybir

    f32 = mybir.dt.float32
    f32r = mybir.dt.float32r
    bf16 = mybir.dt.bfloat16
    AF = mybir.ActivationFunctionType
    ALU = mybir.AluOpType
    AX = mybir.AxisListType

    def r(ap):
        return ap.bitcast(f32r)

    nc = bacc.Bacc(target_bir_lowering=False, debug=False)
    _lp = nc.allow_low_precision(reason="f32r is 4-byte storage, not low precision")
    _lp.__enter__()

    xin_d = {
        t: nc.declare_dram_parameter(t, [128, NT * DIM], bf16, isOutput=False)
        for t in ("xk", "xq", "xv")
    }
    wk_d = nc.declare_dram_parameter("wk", [128, HPG * DIM], bf16, isOutput=False)
    cst_d = nc.declare_dram_parameter("cst", [128, 769], f32, isOutput=False)
    wv_d = nc.declare_dram_parameter("wv", [128, CC * IG], bf16, isOutput=False)
    wo_d = nc.declare_dram_parameter("wo", [128, 2 * DIM], f32, isOutput=False)
    ident_d = nc.declare_dram_parameter("ident", [128, 128], bf16, isOutput=False)
    if has_bias:
        bq_d = nc.declare_dram_parameter("bq", [1, IG], f32, isOutput=False)
        bk_d = nc.declare_dram_parameter("bk", [1, HPG * 128], f32, isOutput=False)
    out_d = nc.declare_dram_parameter("out", [128, NT * DIM], f32, isOutput=True)

    # engine rotation for bulk copies: scalar (Act) / vector (DVE) / gpsimd
    rot = {"i": 0}

    def copy_rr(dst, src, seq=(0, 1)):
        e = seq[rot["i"] % len(seq)]
        rot["i"] += 1
        if e == 0:
            nc.scalar.activation(dst, src, AF.Copy)
        elif e == 1:
            nc.vector.tensor_copy(dst, src)
        else:
            nc.gpsimd.tensor_copy(dst, src)

    with tile.TileContext(nc) as tc, \
         tc.tile_pool(name="persist", bufs=1) as P:

        # ---- persistent constants (DMAs ordered after xk below) ----
        ident_sb = P.tile([128, 128], bf16, name="ident_sb")
        cst = P.tile([128, 769], f32r, name="cst")
        browq = cst[:, 0:128]
        browk = cst[:, 128:256]
        onescol = cst[:, 256:257]
        ones_row = cst[0:1, 257:769]
        eps_sb = P.tile([128, 1], f32, name="eps_sb")
        nc.vector.memset(eps_sb, LN_EPS)
        vwcol = P.tile([97, 1], f32, name="vwcol")
        nc.vector.memset(vwcol, var_w)

        # ---- persistent weights ----
        wv_sb = P.tile([128, CC * IG], bf16, name="wv_sb")
        wo_sb = P.tile([128, 2 * DIM], f32r, name="wo_sb")
        if has_bias:
            bq_sb = P.tile([1, IG], f32r, name="bq_sb")
            nc.sync.dma_start(out=bq_sb, in_=bq_d[:, :].bitcast(f32r))
            bk_sb = P.tile([1, HPG * 128], f32r, name="bk_sb")
            nc.sync.dma_start(out=bk_sb, in_=bk_d[:, :].bitcast(f32r))

        # ---- persistent activations ----
        # L[h]: rows 0:64 kc_h, rows 64:128 fk_h -> fkn_h   [128, N]
        # R[h]: rows 0:64 fqc_h, rows 64:128 fq_h -> fqn_h  [128, N]
        L = [P.tile([128, N], f32r, name=f"L{h}") for h in range(HPG)]
        R = [P.tile([128, N], f32r, name=f"R{h}") for h in range(HPG)]
        fv = [P.tile([128, IG], f32r, name=f"fv{mt}") for mt in range(NT)]
        oT = [P.tile([128, N], f32r, name=f"oT{j}") for j in range(2)]
        # stat rows at partition 32h: qstat = cos_w-ready 1/qn, kstat = 1/kn
        qstat = P.tile([97, N], f32r, name="qstat")
        kstat = P.tile([97, N], f32r, name="kstat")
        vrr = P.tile([97, N], f32r, name="vrr")
        fkst = P.tile([128, HPG], f32r, name="fkst")
        frep = P.tile([97, IG], f32r, name="frep")

        zst = P.tile([97, N], f32, name="zst")
        nc.vector.memset(zst, 1.0)
        nc.scalar.activation(qstat, zst, AF.Copy)
        nc.scalar.activation(kstat, zst, AF.Copy)

        # ======== stages A+B: load, LN, transpose, project ========
        with tc.tile_pool(name="xa", bufs=1) as XA, \
             tc.tile_pool(name="zt", bufs=4) as ZT, \
             tc.tile_pool(name="sqp", bufs=4) as SQP, \
             tc.tile_pool(name="smal", bufs=2) as SM, \
             tc.tile_pool(name="wkp", bufs=1) as WKP, \
             tc.tile_pool(name="pt", bufs=2, space="PSUM") as PT, \
             tc.tile_pool(name="pb", bufs=2, space="PSUM") as PB, \
             tc.tile_pool(name="pstat", bufs=2, space="PSUM") as PST:

            wk_sb = WKP.tile([128, HPG * DIM], bf16, name="wk_sb")

            xin = {}
            xT = {}
            for t in ("xk", "xq", "xv"):
                xin[t] = XA.tile([128, NT * DIM], bf16, tag=f"xin{t}",
                                 name=f"xin{t}")
                xT[t] = XA.tile([128, CC * N], bf16, tag=f"xT{t}",
                                name=f"xT{t}")
            # DMA issue order = transfer order on the DMA engines: get xk in
            # first, then what stage A/B-k needs, then the rest.
            for hf in range(4):
                cs = slice(hf * (NT * DIM // 4), (hf + 1) * (NT * DIM // 4))
                nc.sync.dma_start(out=xin["xk"][:, cs], in_=xin_d["xk"][:, cs])
            nc.sync.dma_start(out=ident_sb, in_=ident_d[:, :])
            for hf in range(2):
                cs = slice(hf * (NT * DIM // 2), (hf + 1) * (NT * DIM // 2))
                nc.sync.dma_start(out=xin["xq"][:, cs], in_=xin_d["xq"][:, cs])
            nc.sync.dma_start(out=wk_sb, in_=wk_d[:, :])
            nc.sync.dma_start(out=cst, in_=cst_d[:, :].bitcast(f32r))
            nc.sync.dma_start(out=wv_sb, in_=wv_d[:, :])
            for hf in range(2):
                cs = slice(hf * (NT * DIM // 2), (hf + 1) * (NT * DIM // 2))
                nc.sync.dma_start(out=xin["xv"][:, cs], in_=xin_d["xv"][:, cs])
            nc.sync.dma_start(out=wo_sb, in_=wo_d[:, :].bitcast(f32r))

            # --- stage A per tensor: LN stats (half-batched), LN, transpose
            for t in ("xk", "xq", "xv"):
                mvt = SM.tile([128, 2 * NT], f32, tag="mvt", name="mvt")
                stt = SQP.tile([128, nc.vector.BN_STATS_DIM], f32, tag="bns", name="stt")
                mv3 = mvt.rearrange("p (t s) -> p t s", s=2)
                rin = SM.tile([128, NT], f32, tag="rin", name="rin")
                nmr = SM.tile([128, NT], f32, tag="nmr", name="nmr")
                xT3 = xT[t].rearrange("p (c n) -> p c n", c=CC)
                for half in range(2):
                    hsl = slice(4 * half, 4 * half + 4)
                    for nt in range(4 * half, 4 * half + 4):
                        nc.vector.bn_stats(
                            out=stt, in_=xin[t][:, nt * DIM:(nt + 1) * DIM])
                        nc.vector.bn_aggr(out=mvt[:, 2 * nt:2 * nt + 2],
                                          in_=stt)
                    nc.scalar.activation(rin[:, hsl], mv3[:, hsl, 1:2],
                                         AF.Sqrt, bias=eps_sb)
                    nc.vector.reciprocal(rin[:, hsl], rin[:, hsl])
                    nc.vector.tensor_tensor(nmr[:, hsl], mv3[:, hsl, 0:1],
                                            rin[:, hsl], ALU.mult)
                    nc.vector.tensor_scalar_mul(nmr[:, hsl], nmr[:, hsl],
                                                -1.0)
                    for nt in range(4 * half, 4 * half + 4):
                        zt = ZT.tile([128, DIM], bf16, tag="zt", name="zt")
                        nc.gpsimd.tensor_scalar(
                            zt, xin[t][:, nt * DIM:(nt + 1) * DIM],
                            rin[:, nt:nt + 1], nmr[:, nt:nt + 1],
                            ALU.mult, ALU.add)
                        pt = PT.tile([128, 512], bf16, tag="pt", name="pt")
                        for c in range(CC):
                            nc.tensor.transpose(
                                pt[:, c * 128:(c + 1) * 128],
                                zt[:, c * 128:(c + 1) * 128], ident_sb)
                        if nt % 2 == 0:
                            nc.vector.tensor_copy(
                                xT3[:, :, nt * 128:(nt + 1) * 128], pt)
                        else:
                            nc.scalar.activation(
                                xT3[:, :, nt * 128:(nt + 1) * 128], pt,
                                AF.Copy)

            # --- stage B-k: aug projection -> L, stats (kstat rows hold kn)
            for h in range(HPG):
                for ncx in range(NC):
                    cs = slice(ncx * 512, (ncx + 1) * 512)
                    pf = PB.tile([128, 512], f32, tag="pf", name="pf")
                    for c in range(CC):
                        nc.tensor.matmul(
                            pf, wk_sb[:, h * DIM + c * 128:h * DIM + (c + 1) * 128],
                            xT["xk"][:, c * N + ncx * 512:c * N + (ncx + 1) * 512],
                            start=(c == 0), stop=(c == 3 and not has_bias))
                    if has_bias:
                        nc.tensor.matmul(
                            pf, r(bk_sb[0:1, h * 128:(h + 1) * 128]),
                            r(ones_row[0:1, :]), start=False, stop=True)
                    copy_rr(L[h][:, cs], pf, seq=(0,))
                    sq = SQP.tile([64, 512], f32r, tag="sq", name="sq")
                    nc.gpsimd.tensor_tensor(sq, L[h][64:128, cs],
                                            L[h][64:128, cs], ALU.mult)
                    ps1 = PST.tile([1, 512], f32, tag="ps", name="ps1")
                    nc.tensor.matmul(ps1, r(onescol[0:64, :]), r(sq),
                                     start=True, stop=True)
                    nc.scalar.activation(kstat[32 * h:32 * h + 1, cs], ps1,
                                         AF.Sqrt)
                if h == 1:
                    nc.vector.reciprocal(kstat[0:33, :], kstat[0:33, :])
            nc.vector.reciprocal(kstat[64:97, :], kstat[64:97, :])

            # --- stage B-q: plain projection (shared weights wv) -> R, stats
            for hp in range(2):
                for ncx in range(NC):
                    cs = slice(ncx * 512, (ncx + 1) * 512)
                    pf = PB.tile([128, 512], f32, tag="pf", name="pf")
                    for c in range(CC):
                        nc.tensor.matmul(
                            pf, wv_sb[:, c * IG + hp * 128:c * IG + (hp + 1) * 128],
                            xT["xq"][:, c * N + ncx * 512:c * N + (ncx + 1) * 512],
                            start=(c == 0), stop=(c == 3 and not has_bias))
                    if has_bias:
                        nc.tensor.matmul(
                            pf, r(bq_sb[0:1, hp * 128:(hp + 1) * 128]),
                            r(ones_row[0:1, :]), start=False, stop=True)
                    for j in range(2):
                        h = 2 * hp + j
                        fq = pf[j * 64:(j + 1) * 64, :]
                        nc.scalar.mul(R[h][0:64, cs], fq,
                                      cov_w / DIM_HEAD)
                        copy_rr(R[h][64:128, cs], fq, seq=(0, 1))
                        sq = SQP.tile([64, 512], f32r, tag="sq", name="sq")
                        nc.gpsimd.tensor_tensor(sq, R[h][64:128, cs],
                                                R[h][64:128, cs], ALU.mult)
                        ps1 = PST.tile([1, 512], f32, tag="ps", name="ps1")
                        nc.tensor.matmul(ps1, r(onescol[0:64, :]), r(sq),
                                         start=True, stop=True)
                        nc.scalar.activation(qstat[32 * h:32 * h + 1, cs],
                                             ps1, AF.Sqrt)
                if hp == 0:
                    nc.vector.reciprocal(qstat[0:33, :], qstat[0:33, :])
            nc.vector.reciprocal(qstat[64:97, :], qstat[64:97, :])

            # --- stage B-v: projection -> fv (n-major), fvsum
            pfs = PST.tile([1, IG], f32, tag="pfs", name="pfs")
            for mt in range(NT):
                pfv = PB.tile([128, 512], f32, tag="pf", name="pfv")[:, 0:IG]
                for c in range(CC):
                    nc.tensor.matmul(
                        pfv, xT["xv"][:, c * N + mt * 128:c * N + (mt + 1) * 128],
                        wv_sb[:, c * IG:(c + 1) * IG],
                        start=(c == 0), stop=(c == 3 and not has_bias))
                if has_bias:
                    nc.tensor.matmul(
                        pfv, r(ones_row[0:1, 0:128]), r(bq_sb[0:1, :]),
                        start=False, stop=True)
                copy_rr(fv[mt], pfv)
            for mt in range(NT):
                nc.tensor.matmul(pfs, r(onescol[:, :]), r(fv[mt]),
                                 start=(mt == 0), stop=(mt == NT - 1))
            for h in range(HPG):
                nc.scalar.activation(frep[32 * h:32 * h + 1, :], pfs, AF.Copy)

        # ======== stage C: normalize L/R, var rows ========
        with tc.tile_pool(name="pbc", bufs=2, space="PSUM") as PBC, \
             tc.tile_pool(name="pvr", bufs=2, space="PSUM") as PVR:
            for h in range(HPG):
                hs = slice(32 * h, 32 * h + 1)
                for ncx in range(NC):
                    cs = slice(ncx * 512, (ncx + 1) * 512)
                    # k side: fkn = fk * bcast(1/kn); q: fqn *= bcast(cos_w/qn)
                    pb = PBC.tile([128, 512], f32, tag="pb", name="pb")
                    nc.tensor.matmul(pb, r(browk[hs, :]), r(kstat[hs, cs]),
                                     start=True, stop=True,
                                     tile_position=(32 * h, 0))
                    nc.vector.tensor_tensor(
                        L[h][64:128, cs], L[h][64:128, cs],
                        pb[64:128, :], ALU.mult)
                    pb2 = PBC.tile([128, 512], f32, tag="pb", name="pb2")
                    nc.tensor.matmul(pb2, r(browq[hs, :]), r(qstat[hs, cs]),
                                     start=True, stop=True,
                                     tile_position=(32 * h, 0))
                    nc.vector.tensor_tensor(
                        R[h][64:128, cs], R[h][64:128, cs],
                        pb2[64:128, :], ALU.mult)
            with tc.tile_pool(name="fksc", bufs=2) as FKS:
                for h in range(HPG):
                    if h % 2 == 0:
                        fscr = FKS.tile([64, N], f32r, tag="fscr",
                                        name="fscr")
                        nc.scalar.activation(fscr, L[h][64:128, :], AF.Copy,
                                             accum_out=fkst[64:128, h:h + 1])
                    else:
                        nc.vector.reduce_sum(fkst[64:128, h:h + 1],
                                             L[h][64:128, :], axis=AX.X)
            # vr rows: vr = var_w - var_w/(N*cos_w) * (fks . fqn)
            for h in range(HPG):
                hs = slice(32 * h, 32 * h + 1)
                for ncx in range(NC):
                    cs = slice(ncx * 512, (ncx + 1) * 512)
                    pv1 = PVR.tile([1, 512], f32, tag="pvr", name="pv1")
                    nc.tensor.matmul(
                        pv1, r(fkst[64:128, h:h + 1]),
                        r(R[h][64:128, cs]), start=True, stop=True)
                    nc.scalar.activation(
                        vrr[hs, cs], pv1, AF.Identity, bias=vwcol[0:1, :],
                        scale=-(var_w / (N * cos_w)))

        # ======== stage D: scores + out-stage (ncx outer) + stage E ========
        with tc.tile_pool(name="pss", bufs=6, space="PSUM") as PSS, \
             tc.tile_pool(name="pop", bufs=2, space="PSUM") as POP, \
             tc.tile_pool(name="obp", bufs=3) as OBP, \
             tc.tile_pool(name="stp", bufs=6) as STP:
            for ncx in range(NC):
                cs = slice(ncx * 512, (ncx + 1) * 512)
                for h in range(HPG):
                    j2, jj = h // 2, h % 2
                    hs = slice(32 * h, 32 * h + 1)
                    po = POP.tile([64, 512], f32, tag="po", name=f"po{h}_{ncx}")
                    for mt in range(NT):
                        ms = slice(mt * 128, (mt + 1) * 128)
                        pss = PSS.tile([128, 512], f32, tag="pss", name="pss")
                        nc.tensor.matmul(pss, r(L[h][:, ms]), r(R[h][:, cs]),
                                         start=True, stop=True)
                        st = STP.tile([128, 512], f32r, tag="st", name="st")
                        copy_rr(st, pss, seq=(0, 1))
                        nc.tensor.matmul(
                            po, r(fv[mt][:, h * 64:(h + 1) * 64]), r(st),
                            start=(mt == 0), stop=False)
                    nc.tensor.matmul(
                        po, r(frep[hs, h * 64:(h + 1) * 64]),
                        r(vrr[hs, cs]), start=False, stop=True,
                        tile_position=(32 * h, 0))
                    copy_rr(oT[j2][jj * 64:(jj + 1) * 64, cs], po)
                # ---- stage E for this n-chunk ----
                for nt in range(4 * ncx, 4 * ncx + 4):
                    pf = PSS.tile([128, 512], f32, tag="pss", name="pfe")
                    for j2 in range(2):
                        nc.tensor.matmul(
                            pf, r(oT[j2][:, nt * 128:(nt + 1) * 128]),
                            r(wo_sb[:, j2 * 512:(j2 + 1) * 512]),
                            start=(j2 == 0), stop=(j2 == 1))
                    obt = OBP.tile([128, DIM], f32, tag="ob", name="obt")
                    copy_rr(obt, pf)
                    nc.sync.dma_start(
                        out=out_d[:, nt * DIM:(nt + 1) * DIM], in_=obt)

    _lp.__exit__(None, None, None)
    nc.compile()
    return nc


def _host_prep_weights(ln_g, ln_b, W_in, W_out, g):
    """Per-head-group weight layouts (see _build_nc docstring)."""
    W_f = (ln_g[:, None] * W_in)[:, g * IG:(g + 1) * IG]  # [512, 256]
    C = np.eye(DIM_HEAD, dtype=np.float32) - 1.0 / DIM_HEAD

    # k-aug per head: [W_h @ C | W_h] -> [512, 128] each
    wk = np.empty((DIM, HPG * 128), np.float32)
    for h in range(HPG):
        Wh = W_f[:, h * 64:(h + 1) * 64]
        wk[:, h * 128:h * 128 + 64] = Wh @ C
        wk[:, h * 128 + 64:(h + 1) * 128] = Wh
    # c-major SBUF layouts: [p, h*512 + c*128 + i] = wk[c*128+p, h*128+i]
    wk_sb = np.ascontiguousarray(
        wk.reshape(CC, 128, HPG, 128).transpose(1, 2, 0, 3).reshape(128, HPG * DIM))
    wv_sb = np.ascontiguousarray(
        W_f.reshape(CC, 128, IG).transpose(1, 0, 2).reshape(128, CC * IG))
    Wo = W_out[g * IG:(g + 1) * IG, :]  # [256, 512]
    wo_sb = np.ascontiguousarray(
        Wo.reshape(2, 128, DIM).transpose(1, 0, 2).reshape(128, 2 * DIM))

    bW = (ln_b @ W_in)[g * IG:(g + 1) * IG].astype(np.float32)  # [256]
    has_bias = bool(np.any(bW))
    bq = bW[None, :]
    bk = np.empty((1, HPG * 128), np.float32)
    for h in range(HPG):
        bh = bW[h * 64:(h + 1) * 64]
        bk[0, h * 128:h * 128 + 64] = bh @ C
        bk[0, h * 128 + 64:(h + 1) * 128] = bh
    return wk_sb, wv_sb, wo_sb, bq, bk, has_bias


def _prep(q, k, v, ln_g, ln_b, W_in, W_out, b_out, cov_w_raw, var_w_raw):
    q = np.asarray(q, np.float32)
    k = np.asarray(k, np.float32)
    v = np.asarray(v, np.float32)
    ln_g = np.asarray(ln_g, np.float32)
    ln_b = np.asarray(ln_b, np.float32)
    W_in = np.asarray(W_in, np.float32)
    W_out = np.asarray(W_out, np.float32)

    cov_w = float(1.0 / (1.0 + np.exp(-np.float64(cov_w_raw))))
    var_w = float(1.0 / (1.0 + np.exp(-np.float64(var_w_raw))))
    cos_w = 1.0 - cov_w - var_w

    per_g = [_host_prep_weights(ln_g, ln_b, W_in, W_out, g) for g in range(HG)]
    has_bias = any(pg[5] for pg in per_g)
    nc = _build_nc(cos_w, cov_w, var_w, has_bias)

    ident = np.eye(128, dtype=np.float32).astype(ml_dtypes.bfloat16)
    cst = np.zeros((128, 769), np.float32)
    for h in range(HPG):
        cst[32 * h, 64:128] = cos_w      # browq (mult by cos_w/qn)
        cst[32 * h, 128 + 64:128 + 128] = 1.0  # browk (divide by kn)
    cst[:, 256] = 1.0                    # onescol
    cst[0, 257:769] = 1.0                # ones_row

    def pmaj(x2d):  # [1024, 512] -> [128, 8*512] p-major, bf16
        return np.ascontiguousarray(
            x2d.reshape(NT, 128, DIM).transpose(1, 0, 2).reshape(
                128, NT * DIM).astype(ml_dtypes.bfloat16))

    in_maps = []
    for core in range(8):
        b, g = core // HG, core % HG
        wk_sb, wv_sb, wo_sb, bq, bk, _ = per_g[g]
        m = {
            "xq": pmaj(q[b]), "xk": pmaj(k[b]), "xv": pmaj(v[b]),
            "wk": wk_sb.astype(ml_dtypes.bfloat16),
            "wv": wv_sb.astype(ml_dtypes.bfloat16),
            "wo": wo_sb, "ident": ident,
            "cst": cst,
        }
        if has_bias:
            m["bq"] = bq
            m["bk"] = bk
        in_maps.append(m)
    return nc, in_maps


def kernel(q, k, v, ln_g, ln_b, W_in, W_out, b_out, cov_w_raw, var_w_raw):
    from concourse.bass_utils import run_bass_kernel_spmd

    b_out = np.asarray(b_out, np.float32)
    nc, in_maps = _prep(q, k, v, ln_g, ln_b, W_in, W_out, b_out,
                        cov_w_raw, var_w_raw)
    res = run_bass_kernel_spmd(nc, in_maps, list(range(8)))

    def unpmaj(o):  # [128, 8*512] -> [1024, 512]
        return o.reshape(128, NT, DIM).transpose(1, 0, 2).reshape(N, DIM)

    parts = [unpmaj(res.results[c]["out"]) for c in range(8)]
    out = np.stack([parts[2 * b] + parts[2 * b + 1] + b_out
                    for b in range(B)])
    return out.astype(np.float32)
